# revision 14
# baseline (speedup 1.0000x reference)
"""Trainium2 Bass kernel for ConvPosDivMultiHeadAttn (B=8, L=512, D=1024, H=16).

Sharding: pure data-parallel over batch — 8 cores, 1 batch element each, all
16 heads on-core, weights replicated. No collectives.

Dtypes: fp16 operands for all projection/score matmuls (full PE rate, 11-bit
mantissa), fp32 PSUM accumulation, bf16 for exp outputs / V (needs dynamic
range up to e^~35), fp32 final output.

Per-core pipeline:
  1. x (fp16) -> SBUF per 128-token block, transpose to xT [d, t] via PE.
     Weights land via 7 large DMAs issued in consumption order.
  2. Feature-major q/k projection psum[f, t] = w-slice-as-lhsT @ xT, plain
     PSUM->SBUF fp16 copies on the Activation engine (2 heads per tile).
  3. Speaker-identity masking via the +/-1 trick:
       qsame[i,j] = (1 + t_i t_j)/2,  t = 2*qmask - 1 in {-1,+1}
     so  (qk + pos) * qsame = [k;kp]^T [q/2;qp/2] + (t.[k;kp])^T (t.[q/2;qp/2])
     Per head, DVE merges build packed 128-row operands KS/QS (k stacked on
     kp) and their t-scaled twins KT/QT — all fp16 SBUF ops at 2x DVE rate.
  4. v projected token-major into a ones-augmented V (extra column of 1s per
     head) so the AV matmul also yields the softmax denominator.
  5. Scores TRANSPOSED per (head, j-tile): 3 accumulating matmuls
     (KS.QS + KT.QT + I @ A'^T) where A'^T is host-precomputed gaussian bias
     + key padding - row stabilizer. exp on Activation (PSUM -> bf16 SBUF).
  6. out^T[d, i] (+ sums row) = V_aug-as-lhsT @ E^T; normalization via DVE
     reciprocal of the sums row, K=1 matmul broadcast, Pool-engine staging
     copy, DVE multiply into out_attn^T (fp16, feature-major).
     AV/normalize for head h are emitted after head h+1's scores (software
     pipelining) so the in-order PE queue never stalls on the exp round-trip.
  7. y = out_attn^T-as-lhsT @ w_fc + b_fc (fp32 out, bias fused).
"""

import sys

import ml_dtypes
import numpy as np

sys.path.insert(0, "/opt/trn_rl_repo")

import concourse.bass as bass  # noqa: E402
import concourse.tile as tile  # noqa: E402
from concourse import bacc, mybir  # noqa: E402
from concourse.masks import make_identity  # noqa: E402

B, L, D, H = 8, 512, 1024, 16
HD = D // H  # 64
FP = mybir.dt.float32
F16 = mybir.dt.float16
BF = mybir.dt.bfloat16


def build_kernel(nc):
    """Emit the single-core program. All loops static/unrolled under Tile."""
    from contextlib import ExitStack

    AF = mybir.ActivationFunctionType
    OP = mybir.AluOpType

    x = nc.dram_tensor("x", [L, D], F16, kind="ExternalInput").ap()
    wqkv = nc.dram_tensor("wqkv", [D, 3 * D], F16, kind="ExternalInput").ap()
    wqp = nc.dram_tensor("wqp", [HD, 2 * D], F16, kind="ExternalInput").ap()
    wfc = nc.dram_tensor("wfc", [D, D], F16, kind="ExternalInput").ap()
    peT = nc.dram_tensor("peT", [HD, L], F16, kind="ExternalInput").ap()
    ea = nc.dram_tensor("ea", [L, L], BF, kind="ExternalInput").ap()
    T16 = nc.dram_tensor("T16", [128, L], F16, kind="ExternalInput").ap()
    BB = nc.dram_tensor("BB", [128, D], FP, kind="ExternalInput").ap()
    y = nc.dram_tensor("y", [L, D], FP, kind="ExternalOutput").ap()

    with tile.TileContext(nc) as tc:
        with ExitStack() as ctx:
            ctx.enter_context(
                nc.allow_low_precision(reason="fp16/bf16 operand pipeline by design")
            )
            const = ctx.enter_context(tc.tile_pool(name="const", bufs=1))
            wpool = ctx.enter_context(tc.tile_pool(name="wp", bufs=1))
            xpool = ctx.enter_context(tc.tile_pool(name="xp", bufs=1))
            big = ctx.enter_context(tc.tile_pool(name="big", bufs=1))
            qksb = ctx.enter_context(tc.tile_pool(name="qksb", bufs=8))
            possb = ctx.enter_context(tc.tile_pool(name="possb", bufs=16))
            qkop = ctx.enter_context(tc.tile_pool(name="qkop", bufs=4))
            atp = ctx.enter_context(tc.tile_pool(name="atp", bufs=4))
            etp = ctx.enter_context(tc.tile_pool(name="etp", bufs=12))
            ysb = ctx.enter_context(tc.tile_pool(name="ysb", bufs=4))
            rcp = ctx.enter_context(tc.tile_pool(name="rcp", bufs=4))
            pp = ctx.enter_context(tc.tile_pool(name="pp", bufs=2, space="PSUM"))
            sp = ctx.enter_context(tc.tile_pool(name="sp", bufs=3, space="PSUM"))
            ap_ = ctx.enter_context(tc.tile_pool(name="ap", bufs=2, space="PSUM"))
            rp = ctx.enter_context(tc.tile_pool(name="rp", bufs=1, space="PSUM"))

            # ---- phase 0: DMAs, issued in consumption order ----
            # x token blocks, transposed on PE into xT while weights stream
            # (PE and Act are otherwise idle during the weight DMAs)
            xtok = []
            for tc_ in range(4):
                xt_b = xpool.tile([128, D], F16, name=f"xtok{tc_}")
                nc.sync.dma_start(xt_b[:], x[tc_ * 128 : (tc_ + 1) * 128, :])
                xtok.append(xt_b)
            xT = xpool.tile([128, 8 * 512], F16)

            def wslice(dst, col0, ncol):
                src = wqkv[:, col0 : col0 + ncol].rearrange(
                    "(k p) c -> p k c", p=128
                )
                nc.sync.dma_start(
                    dst[:].rearrange("p (k c) -> p k c", c=ncol), src
                )

            wq_sb, wk_sb = [None, None], [None, None]
            wq_sb[0] = wpool.tile([128, 8 * 512], F16, name="wq0")
            wslice(wq_sb[0], 0, 512)
            wk_sb[0] = wpool.tile([128, 8 * 512], F16, name="wk0")
            wslice(wk_sb[0], D, 512)

            ident = const.tile([128, 128], F16)
            make_identity(nc, ident[:])
            ones64 = const.tile([128, 64], F16)
            nc.vector.memset(ones64[:], 1.0)
            ones128 = const.tile([1, 128], F16)
            nc.vector.memset(ones128[:], 1.0)
            bbrow = const.tile([1, D], F16)
            tb = const.tile([128, L], F16)
            nc.sync.dma_start(tb[:], T16)
            pet = const.tile([HD, L], F16)
            nc.sync.dma_start(pet[:], peT)
            wqpt = const.tile([HD, 2 * D], F16)
            nc.sync.dma_start(wqpt[:], wqp)

            wv_sb = []
            for nv in range(2):
                wv = wpool.tile([128, 8 * 512], F16, name=f"wv{nv}")
                wslice(wv, 2 * D + nv * 512, 512)
                wv_sb.append(wv)

            ea_sb = []
            for jt in range(4):
                a = atp.tile([128, 512], BF)
                nc.sync.dma_start(a[:], ea[jt * 128 : (jt + 1) * 128, :])
                ea_sb.append(a)

            wq_sb[1] = wpool.tile([128, 8 * 512], F16, name="wq1")
            wslice(wq_sb[1], 512, 512)
            wk_sb[1] = wpool.tile([128, 8 * 512], F16, name="wk1")
            wslice(wk_sb[1], D + 512, 512)

            wfc_sb = wpool.tile([128, 8 * 1024], F16)
            nc.sync.dma_start(
                wfc_sb[:].rearrange("p (k c) -> p k c", c=1024),
                wfc[:, :].rearrange("(k p) c -> p k c", p=128),
            )
            bbt = const.tile([128, D], FP)
            nc.sync.dma_start(bbt[:], BB)
            nc.vector.tensor_copy(bbrow[:], bbt[0:1, :])

            # build xT [d, t]: PE transposes per x block, Act PSUM->SBUF copies
            xT3 = xT[:].rearrange("p (d c) -> p d c", c=512)
            for tc_ in range(4):
                for half in range(2):
                    pb = pp.tile([128, 512], F16, tag="pp")
                    for dq in range(4):
                        dc = half * 4 + dq
                        nc.tensor.transpose(
                            pb[:, dq * 128 : (dq + 1) * 128],
                            xtok[tc_][:, dc * 128 : (dc + 1) * 128],
                            ident[:],
                        )
                    pb3 = pb[:].rearrange("p (d c) -> p d c", c=128)
                    nc.scalar.copy(
                        xT3[:, half * 4 : (half + 1) * 4, tc_ * 128 : tc_ * 128 + 128],
                        pb3[:],
                    )

            oaT = big.tile([128, 8 * 512], F16)  # out_attn^T, feature-major
            vaug = big.tile([128, 4 * 16 * 65], BF)
            v3 = vaug[:].rearrange("p (c e) -> p c e", e=65)
            nc.vector.memset(v3[:, :, 64:65], 1.0)

            # ---- per-group projections + software-pipelined head loop ----
            qk_q = [None] * 4  # per group: q psum->sbuf tiles (2 heads each)
            qk_k = [None] * 4
            ops_d = {}
            ets_d = {}
            av_d = {}
            rec_d = {}

            pos_all = {}

            def pos_group(g):
                # positional projection (feature-major), batch-independent
                for fpt in range(8):
                    isq = fpt < 4
                    p = fpt if isq else fpt - 4
                    col = (0 if isq else D) + g * 512 + p * 128
                    yp = pp.tile([128, 512], FP, tag="pp")
                    nc.tensor.matmul(
                        yp[:], wqpt[:, col : col + 128], pet[:],
                        start=True, stop=True,
                    )
                    dst = possb.tile([128, 512], F16, tag="possb")
                    nc.scalar.copy(dst[:], yp[:])
                    pos_all[(g, isq, p)] = dst

            def proj_group(g, sides=(True, False)):
                # q/k projection (feature-major), Act plain copies to fp16
                for fpt in range(8):
                    isq = fpt < 4
                    if isq not in sides:
                        continue
                    p = fpt if isq else fpt - 4
                    wsb = wq_sb[g] if isq else wk_sb[g]
                    qp_ps = pp.tile([128, 512], FP, tag="pp")
                    for kc in range(8):
                        nc.tensor.matmul(
                            qp_ps[:],
                            wsb[:, kc * 512 + p * 128 : kc * 512 + p * 128 + 128],
                            xT[:, kc * 512 : (kc + 1) * 512],
                            start=(kc == 0),
                            stop=(kc == 7),
                        )
                    dst = qksb.tile([128, 512], F16, tag="qksb")
                    nc.scalar.copy(dst[:], qp_ps[:])
                    (qk_q if isq else qk_k)[p] = dst


            def stage_merge(h):
                # build packed 128-row operands for head h (fp16 SBUF, DVE 2x)
                hl = h % 8
                hb = (hl % 2) * 64
                p = hl // 2
                g = h // 8
                QS = qkop.tile([128, 512], F16, tag="QS")
                nc.vector.tensor_scalar_mul(QS[0:64, :], qk_q[p][hb : hb + 64, :], 0.5)
                nc.vector.tensor_scalar_mul(
                    QS[64:128, :], pos_all[(g, True, p)][hb : hb + 64, :], 0.5
                )
                QT = qkop.tile([128, 512], F16, tag="QT")
                nc.vector.tensor_mul(QT[:], QS[:], tb[:])
                KS = qkop.tile([128, 512], F16, tag="KS")
                nc.gpsimd.tensor_copy(KS[0:64, :], qk_k[p][hb : hb + 64, :])
                nc.gpsimd.tensor_copy(KS[64:128, :], pos_all[(g, False, p)][hb : hb + 64, :])
                KT = qkop.tile([128, 512], F16, tag="KT")
                nc.vector.tensor_mul(KT[:], KS[:], tb[:])
                ops_d[h] = (QS, QT, KS, KT)

            def stage_scores(h):
                QS, QT, KS, KT = ops_d.pop(h)
                ets = []
                for jt in range(4):
                    s_ps = sp.tile([128, 512], FP, tag="sp")
                    jsl = slice(jt * 128, jt * 128 + 128)
                    nc.tensor.matmul(
                        s_ps[:], KS[:, jsl], QS[:], start=True, stop=False
                    )
                    nc.tensor.matmul(
                        s_ps[:], KT[:, jsl], QT[:], start=False, stop=True
                    )
                    e_t = etp.tile([128, 512], BF)
                    nc.scalar.activation(e_t[:], s_ps[:], AF.Exp)
                    e2 = etp.tile([128, 512], BF, tag="e2")
                    eng = nc.gpsimd if jt == 1 else nc.vector
                    eng.tensor_mul(e2[:], e_t[:], ea_sb[jt][:])
                    ets.append(e2)
                ets_d[h] = ets

            def stage_av(h):
                ets = ets_d.pop(h)
                av = ap_.tile([128, 512], FP, tag="ap")
                for jt in range(4):
                    base = jt * 16 * 65 + h * 65
                    nc.tensor.matmul(
                        av[0:65, :],
                        vaug[:, base : base + 65],
                        ets[jt][:],
                        start=(jt == 0),
                        stop=(jt == 3),
                    )
                rec = rcp.tile([128, 512], F16, tag="rec")
                nc.vector.reciprocal(rec[64:65, :], av[64:65, :])
                av_d[h] = av
                rec_d[h] = rec

            def stage_norm(h):
                av = av_d.pop(h)
                rec = rec_d.pop(h)
                rb = rp.tile([64, 512], FP, tag="rp")
                nc.tensor.matmul(
                    rb[:], ones64[64:65, 0:64], rec[64:65, :],
                    start=True, stop=True,
                )
                rbs = rcp.tile([64, 512], FP, tag="rbs")
                nc.vector.tensor_copy(rbs[:], rb[:])
                ob = (h % 2) * 64
                op_ = (h // 2) * 512
                nc.vector.tensor_mul(
                    oaT[ob : ob + 64, op_ : op_ + 512], av[0:64, :], rbs[:]
                )

            def vproj(nv):
                for tc_ in range(4):
                    vp = pp.tile([128, 512], FP, tag="pp")
                    for kc in range(8):
                        nc.tensor.matmul(
                            vp[:],
                            xT[:, kc * 512 + tc_ * 128 : kc * 512 + tc_ * 128 + 128],
                            wv_sb[nv][:, kc * 512 : (kc + 1) * 512],
                            start=(kc == 0),
                            stop=(kc == 7),
                        )
                    eng = nc.vector if nv == 0 else nc.scalar
                    eng_copy = (
                        nc.vector.tensor_copy if nv == 0 else nc.scalar.copy
                    )
                    eng_copy(
                        v3[:, tc_ * 16 + nv * 8 : tc_ * 16 + (nv + 1) * 8, 0:64],
                        vp[:].rearrange("p (a b) -> p a b", b=64),
                    )

            for h in range(H + 2):
                if h < H:
                    if h == 0:
                        proj_group(0)
                        vproj(0)
                        pos_group(0)
                        stage_merge(0)
                    elif h == 2:
                        vproj(1)
                    elif h == 4:
                        pos_group(1)
                    if h % 8 == 0:
                        stage_merge(h)
                    stage_merge(h + 1) if (h % 8) < 7 and h + 1 < H else None
                    if h == 6:
                        proj_group(1, sides=(True,))
                    elif h == 7:
                        proj_group(1, sides=(False,))
                    stage_scores(h)
                if h >= 1 and h - 1 < H:
                    stage_av(h - 1)
                if h >= 2:
                    stage_norm(h - 2)

            # ---- phase 8: FC + bias ----
            for ne in range(2):
                for tc_ in range(4):
                    yp_ = ap_.tile([128, 512], FP, tag="ap")
                    for fc8 in range(8):
                        nc.tensor.matmul(
                            yp_[:],
                            oaT[:, fc8 * 512 + tc_ * 128 : fc8 * 512 + tc_ * 128 + 128],
                            wfc_sb[:, fc8 * 1024 + ne * 512 : fc8 * 1024 + ne * 512 + 512],
                            start=(fc8 == 0),
                            stop=False,
                        )
                    nc.tensor.matmul(
                        yp_[:], ones128[:], bbrow[:, ne * 512 : (ne + 1) * 512],
                        start=False, stop=True,
                    )
                    y_t = ysb.tile([128, 512], FP)
                    nc.scalar.copy(y_t[:], yp_[:])
                    nc.sync.dma_start(
                        y[tc_ * 128 : (tc_ + 1) * 128, ne * 512 : (ne + 1) * 512],
                        y_t[:],
                    )
    return nc


def host_prep(x, mask, qmask, w_qkv, w_qkpos, w_fc, b_fc, shift, bias):
    """Build per-core input maps (host-side numpy only)."""
    x = np.asarray(x, np.float32)
    mask = np.asarray(mask)
    qmask = np.asarray(qmask)
    b_fc = np.asarray(b_fc, np.float32)
    shift = float(np.asarray(shift).reshape(-1)[0])
    bias = float(np.asarray(bias).reshape(-1)[0])
    wqkv16 = np.asarray(w_qkv).astype(np.float16)
    wqp16 = np.asarray(w_qkpos).astype(np.float16)
    wfc16 = np.asarray(w_fc).astype(np.float16)

    half = HD // 2
    inv = np.exp(np.arange(half, dtype=np.float64) * (-(np.log(10000.0) / (half - 1))))
    r = np.arange(-(L // 2), L // 2, dtype=np.float64)
    ang = r[:, None] * inv[None, :]
    pe = np.concatenate([np.sin(ang), np.cos(ang)], axis=1).astype(np.float32)
    peT16 = np.ascontiguousarray(pe.T).astype(np.float16)  # (HD, L)

    idx = np.arange(L, dtype=np.float32)
    sqd = (idx[:, None] - idx[None, :]) ** 2
    G = -(shift * sqd + bias)  # (L, L), symmetric

    BBrow = np.ascontiguousarray(
        np.broadcast_to(b_fc[None, :], (128, D)).astype(np.float32)
    )

    in_maps = []
    for b in range(B):
        kneg = np.where(mask[b] == 0, np.float32(-1.0e9), np.float32(0.0))
        c_base = (G + kneg[None, :]).max(axis=1)  # max over valid j
        aT = (G + kneg[:, None] - c_base[None, :]).astype(np.float64)  # [j, i]
        eaT = np.exp(aT).astype(np.float32)  # in [0, 1]; exact 0 for padded keys
        t = (2.0 * qmask[b] - 1.0).astype(np.float16)
        T16 = np.ascontiguousarray(np.broadcast_to(t[None, :], (128, L)))
        in_maps.append(
            dict(
                x=np.ascontiguousarray(x[b]).astype(np.float16),
                wqkv=wqkv16,
                wqp=wqp16,
                wfc=wfc16,
                peT=peT16,
                ea=np.ascontiguousarray(eaT).astype(ml_dtypes.bfloat16),
                T16=T16,
                BB=BBrow,
            )
        )
    return in_maps


_NC_CACHE = {}


def get_nc():
    if "nc" not in _NC_CACHE:
        nc = bacc.Bacc(
            "TRN2", target_bir_lowering=False, debug=False, enable_asserts=False,
            num_devices=B,
        )
        build_kernel(nc)
        nc.compile()
        _NC_CACHE["nc"] = nc
    return _NC_CACHE["nc"]


def kernel(**inputs):
    from concourse import bass_utils

    in_maps = host_prep(**inputs)
    nc = get_nc()
    res = bass_utils.run_bass_kernel_spmd(nc, in_maps, list(range(B)))
    out = np.stack([m["y"] for m in res.results], axis=0)
    return out.astype(np.float32)


if __name__ == "__main__":
    rng = np.random.default_rng(0)
    ins = dict(
        x=rng.standard_normal((B, L, D), dtype=np.float32),
        mask=rng.integers(0, 2, (B, L)).astype(np.int64),
        qmask=rng.integers(0, 2, (B, L)).astype(np.int64),
        w_qkv=(rng.standard_normal((D, 3 * D), dtype=np.float32) * 0.02),
        w_qkpos=(rng.standard_normal((HD, 2 * D), dtype=np.float32) * 0.02),
        w_fc=(rng.standard_normal((D, D), dtype=np.float32) * 0.02),
        b_fc=np.zeros((D,), np.float32),
        shift=np.abs(rng.standard_normal(1)).astype(np.float32) + 0.001,
        bias=-np.abs(rng.standard_normal(1)).astype(np.float32),
    )
    ins["mask"][:, 0] = 1
    out = kernel(**ins)
    print(out.shape, out.dtype)
